# revision 1
# baseline (speedup 1.0000x reference)
import sys
sys.path.insert(0, "/opt/trn_rl_repo")
import numpy as np
from contextlib import ExitStack

from concourse import bacc, mybir, tile
from concourse import bass_utils
from concourse.masks import make_identity

# nn_MultiHeadAttention: B=4, T=2048, C=1024, H=16, HS=64
# Sharding: core = 2*b + hh; each core handles batch b, heads hh*8..hh*8+7.
# Per-core output is a partial [T, C] (its 8 heads through Wproj rows);
# host sums the pair (hh=0,1) per batch. Bias folded into hh==0 cores.

B, T, C = 4, 2048, 1024
H, HS = 16, 64
HL = 8            # local heads per core
W = HL * HS       # 512
SCALE = 1.0 / 32.0  # C ** -0.5

F32 = mybir.dt.float32
F32R = mybir.dt.float32r
AF = mybir.ActivationFunctionType
ALU = mybir.AluOpType

_NC = {}


def _build(repeat=1, bdve=True):
    nc = bacc.Bacc("TRN2", target_bir_lowering=False, debug=False, num_devices=1)
    xb_ap = nc.dram_tensor("XB", (T, C), F32, kind="ExternalInput").ap()
    wqk_ap = nc.dram_tensor("WQK", (HL * C, 2 * HS), F32, kind="ExternalInput").ap()
    wv_ap = nc.dram_tensor("WV", (C, W), F32, kind="ExternalInput").ap()
    wp_ap = nc.dram_tensor("WP", (W, C), F32, kind="ExternalInput").ap()
    bias_ap = nc.dram_tensor("BIAS", (1, C), F32, kind="ExternalInput").ap()
    amask_ap = nc.dram_tensor("AMASK", (128, 4, 512), F32, kind="ExternalInput").ap()
    out_ap = nc.dram_tensor("OUT", (T, C), F32, kind="ExternalOutput").ap()

    with tile.TileContext(nc) as tc, ExitStack() as ctx:
        pers = ctx.enter_context(tc.tile_pool(name="pers", bufs=1))
        ident_sb = pers.tile([128, 128], F32)
        amask_sb = pers.tile([128, 4, 512], F32R)
        bias_sb = pers.tile([1, C], F32R)
        ones_row = pers.tile([1, 128], F32R)
        xT = pers.tile([128, 8, T], F32R)            # xT[p, ct, t] = x[t, ct*128+p]
        v_sb = pers.tile([128, 16, HL, HS + 1], F32R)  # [t-part, tt, h, d | ones]
        out_T = pers.tile([128, 4, T], F32R)         # attn out^T, cc = ct*128+p

        make_identity(nc, ident_sb)
        nc.scalar.dma_start(amask_sb, amask_ap.bitcast(F32R))
        nc.scalar.dma_start(bias_sb, bias_ap.bitcast(F32R))
        nc.scalar.activation(ones_row, ident_sb[0:1, :], AF.Copy, bias=1.0, scale=0.0)
        # ones column of v (for row-sum l): free sizes match (16*8*1 == 128)
        nc.scalar.activation(v_sb[:, :, :, HS:HS + 1], ident_sb, AF.Copy,
                             bias=1.0, scale=0.0)

        for _rep in range(repeat):
            # ---- Phase 1+2: xT = x^T, then v = x @ Wv ----
            xg_r = xb_ap.rearrange("(g q p) c -> g p q c", g=4, p=128)
            with tc.tile_pool(name="wv", bufs=1) as wv_pool, \
                 tc.tile_pool(name="xg", bufs=2) as xg_pool, \
                 tc.tile_pool(name="ps12", bufs=2, space="PSUM") as ps12:
                wv_sb = wv_pool.tile([128, 8, W], F32R)
                nc.scalar.dma_start(
                    wv_sb, wv_ap.rearrange("(ct p) n -> p ct n", p=128).bitcast(F32R))
                for g in range(4):
                    xg = xg_pool.tile([128, 4, C], F32)
                    nc.sync.dma_start(xg, xg_r[g])
                    for ct in range(8):
                        pt = ps12.tile([128, 512], F32)
                        for q in range(4):
                            nc.tensor.transpose(pt[:, q * 128:(q + 1) * 128],
                                                xg[:, q, ct * 128:(ct + 1) * 128],
                                                ident_sb)
                        nc.scalar.copy(xT[:, ct, g * 512:(g + 1) * 512], pt)
                for tt in range(16):
                    pv_ = ps12.tile([128, W], F32)
                    for ct in range(8):
                        nc.tensor.matmul(pv_, xT[:, ct, tt * 128:(tt + 1) * 128],
                                         wv_sb[:, ct, :],
                                         start=(ct == 0), stop=(ct == 7))
                    nc.scalar.copy(v_sb[:, tt, :, 0:HS], pv_)

            # ---- Phase 3: per-head QK^T, causal softmax, @V (pipelined) ----
            wqk_r = wqk_ap.rearrange("(h ct p) m -> h p ct m", h=HL, p=128)
            DEPTH = 2
            pending = None
            with tc.tile_pool(name="wqk", bufs=2) as wqk_pool, \
                 tc.tile_pool(name="qkt", bufs=1) as qkt_pool, \
                 tc.tile_pool(name="qhi", bufs=4) as qhi_pool, \
                 tc.tile_pool(name="pp", bufs=6) as p_pool, \
                 tc.tile_pool(name="nrm", bufs=2) as n_pool, \
                 tc.tile_pool(name="psqk", bufs=2, space="PSUM") as ps_qk, \
                 tc.tile_pool(name="pss", bufs=3, space="PSUM") as ps_s, \
                 tc.tile_pool(name="pspv", bufs=2, space="PSUM") as ps_pv, \
                 tc.tile_pool(name="psb", bufs=1, space="PSUM") as ps_b:

                def norm_pre(pv):
                    stg = n_pool.tile([HS + 1, 512], F32)
                    nc.scalar.copy(stg, pv)
                    rcp = n_pool.tile([1, 512], F32)
                    nc.vector.reciprocal(rcp, stg[HS:HS + 1, :])
                    rcr = n_pool.tile([1, 512], F32R)
                    nc.scalar.copy(rcr, rcp)
                    return stg, rcr

                def norm_post(stg, rcr, h, tc4):
                    pb = ps_b.tile([HS, 512], F32)
                    nc.tensor.matmul(pb, ones_row[:, 0:HS], rcr,
                                     start=True, stop=True)
                    st2 = n_pool.tile([HS, 512], F32R)
                    nc.vector.scalar_tensor_tensor(st2, stg[0:HS, :], 1.0, pb,
                                                   ALU.mult, ALU.mult)
                    nc.sync.dma_start(
                        out_T[(h % 2) * 64:(h % 2) * 64 + 64, h // 2,
                              tc4 * 512:(tc4 + 1) * 512],
                        st2)

                for h in range(HL):
                    wqk_sb = wqk_pool.tile([128, 8, 2 * HS], F32R)
                    nc.scalar.dma_start(wqk_sb, wqk_r[h].bitcast(F32R))
                    qkT = qkt_pool.tile([128, T], F32R)  # rows 0-63 q^T, 64-127 k^T
                    qhis = []
                    for tc4 in range(4):
                        pqk = ps_qk.tile([128, 512], F32)
                        for ct in range(8):
                            nc.tensor.matmul(pqk, wqk_sb[:, ct, :],
                                             xT[:, ct, tc4 * 512:(tc4 + 1) * 512],
                                             start=(ct == 0), stop=(ct == 7))
                        nc.scalar.copy(qkT[:, tc4 * 512:(tc4 + 1) * 512], pqk)
                        # matmul requires equal base partitions: stage the q chunk
                        # into partitions 64..127 alongside kT (qkT rows 64..127)
                        qhi = qhi_pool.tile([128, 512], F32R)
                        nc.sync.dma_start(qhi[64:128, :],
                                          qkT[0:64, tc4 * 512:(tc4 + 1) * 512])
                        qhis.append(qhi)
                    for tc4 in range(4):
                        n_s = 4 * (tc4 + 1)
                        qhi = qhis[tc4]
                        pv = ps_pv.tile([HS + 1, 512], F32)
                        p_list = []

                        def emit_pv(st, pv=pv, p_list=p_list, n_s=n_s, h=h):
                            nc.tensor.matmul(pv, v_sb[:, st, h, :], p_list[st],
                                             start=(st == 0),
                                             stop=(st == n_s - 1))

                        for st in range(n_s):
                            sps = ps_s.tile([128, 512], F32)
                            nc.tensor.matmul(sps,
                                             qkT[64:128, st * 128:(st + 1) * 128],
                                             qhi[64:128, :],
                                             start=True, stop=True)
                            p_t = p_pool.tile([128, 512], F32R)
                            nc.scalar.activation(p_t, sps, AF.Exp,
                                                 bias=0.0, scale=SCALE)
                            if st >= 4 * tc4:
                                nc.vector.scalar_tensor_tensor(
                                    p_t, p_t, 1.0, amask_sb[:, st - 4 * tc4, :],
                                    ALU.mult, ALU.mult)
                            p_list.append(p_t)
                            if st == 2 and pending is not None:
                                norm_post(*pending)
                                pending = None
                            if st >= DEPTH:
                                emit_pv(st - DEPTH)
                        for st in range(max(n_s - DEPTH, 0), n_s):
                            emit_pv(st)
                        pending = (*norm_pre(pv), h, tc4)
                norm_post(*pending)
                pending = None

            # ---- Phase 4: out = attn^T.T @ Wproj + bias ----
            out_r = out_ap.rearrange("(tt p) n -> p tt n", p=128)
            with tc.tile_pool(name="wp", bufs=1) as wp_pool, \
                 tc.tile_pool(name="ostg", bufs=2) as o_pool, \
                 tc.tile_pool(name="ps4", bufs=4, space="PSUM") as ps4:
                wp_sb = wp_pool.tile([128, 4, C], F32R)
                nc.scalar.dma_start(
                    wp_sb, wp_ap.rearrange("(ct p) n -> p ct n", p=128).bitcast(F32R))
                if bdve:
                    bias_bc = wp_pool.tile([128, C], F32)
                    for ch in range(2):
                        pbb = ps4.tile([128, 512], F32)
                        nc.tensor.matmul(pbb, ones_row,
                                         bias_sb[:, ch * 512:(ch + 1) * 512],
                                         start=True, stop=True)
                        nc.scalar.copy(bias_bc[:, ch * 512:(ch + 1) * 512], pbb)
                for tt in range(16):
                    ostg = o_pool.tile([128, C], F32)
                    for ch in range(2):
                        po = ps4.tile([128, 512], F32)
                        for ct in range(4):
                            nc.tensor.matmul(po, out_T[:, ct, tt * 128:(tt + 1) * 128],
                                             wp_sb[:, ct, ch * 512:(ch + 1) * 512],
                                             start=(ct == 0),
                                             stop=(bdve and ct == 3))
                        if bdve:
                            nc.vector.scalar_tensor_tensor(
                                ostg[:, ch * 512:(ch + 1) * 512], po, 1.0,
                                bias_bc[:, ch * 512:(ch + 1) * 512],
                                ALU.mult, ALU.add)
                        else:
                            nc.tensor.matmul(po, ones_row,
                                             bias_sb[:, ch * 512:(ch + 1) * 512],
                                             start=False, stop=True)
                            nc.scalar.copy(ostg[:, ch * 512:(ch + 1) * 512], po)
                    eng = nc.sync if tt % 2 == 0 else nc.scalar
                    eng.dma_start(out_r[:, tt, :], ostg)

    nc.finalize()
    return nc


def _in_maps(inputs):
    x = np.ascontiguousarray(np.asarray(inputs["x"], dtype=np.float32))
    Wq = np.asarray(inputs["Wq"], dtype=np.float32)
    Wk = np.asarray(inputs["Wk"], dtype=np.float32)
    Wv = np.asarray(inputs["Wv"], dtype=np.float32)
    Wp = np.asarray(inputs["Wproj"], dtype=np.float32)
    bp = np.asarray(inputs["bproj"], dtype=np.float32)

    s = np.arange(128)[:, None, None]
    j = np.arange(4)[None, :, None]
    tf = np.arange(512)[None, None, :]
    amask = np.where(128 * j + s > tf, np.float32(0.0), np.float32(1.0))
    amask = np.ascontiguousarray(amask.astype(np.float32))

    maps = []
    for core in range(8):
        b, hh = core // 2, core % 2
        hs0 = hh * HL
        wqk = np.concatenate([Wq[hs0:hs0 + HL], Wk[hs0:hs0 + HL]], axis=2)
        wqk = np.ascontiguousarray(wqk.reshape(HL * C, 2 * HS))
        wv = np.ascontiguousarray(
            Wv[hs0:hs0 + HL].transpose(1, 0, 2).reshape(C, W))
        wp = np.ascontiguousarray(Wp[hh * W:(hh + 1) * W])
        bias = (bp if hh == 0 else np.zeros_like(bp)).reshape(1, C)
        maps.append({
            "XB": np.ascontiguousarray(x[b]),
            "WQK": wqk, "WV": wv, "WP": wp,
            "BIAS": np.ascontiguousarray(bias),
            "AMASK": amask,
        })
    return maps


def get_nc(repeat=1, bdve=True):
    key = (repeat, bdve)
    if key not in _NC:
        _NC[key] = _build(repeat, bdve)
    return _NC[key]


def run(inputs, trace=False):
    res = bass_utils.run_bass_kernel_spmd(
        get_nc(), _in_maps(inputs), core_ids=list(range(8)), trace=trace)
    outs = [res.results[c]["OUT"] for c in range(8)]
    out = np.stack([outs[2 * b] + outs[2 * b + 1] for b in range(B)])
    return out.astype(np.float32), res.exec_time_ns


def kernel(**inputs):
    return run(inputs, trace=False)[0]



# revision 15
# speedup vs baseline: 1.4793x; 1.4793x over previous
import sys
sys.path.insert(0, "/opt/trn_rl_repo")
import numpy as np
from contextlib import ExitStack

from concourse import bacc, mybir, tile
from concourse import bass_utils

# nn_MultiHeadAttention: B=4, T=2048, C=1024, H=16, HS=64
# Sharding: core = 2*b + hh; each core handles batch b, heads hh*8..hh*8+7.
# Per-core output is a partial [T, C] (its 8 heads through Wproj rows);
# host sums the pair (hh=0,1) per batch. Bias folded into hh==0 cores.
#
# fp16 compute, f32 PSUM accumulation. Attention out kept in natural
# [t, d] layout (PV matmul p.T @ [v|1] with query on partitions), both
# transposes (x^T, attn^T) via XBAR dma_start_transpose, exp merged into
# [128, 2*512] activation instructions, all PSUM->SBUF copies on Pool/DVE.

B, T, C = 4, 2048, 1024
H, HS = 16, 64
HL = 8            # local heads per core
W = HL * HS       # 512
SCALE = 1.0 / 32.0  # C ** -0.5

F32 = mybir.dt.float32
F32R = mybir.dt.float32r
F16 = mybir.dt.float16
AF = mybir.ActivationFunctionType
ALU = mybir.AluOpType

_NC = {}


def _build(repeat=1):
    nc = bacc.Bacc("TRN2", target_bir_lowering=False, debug=False, num_devices=1)
    xb_ap = nc.dram_tensor("XB", (T, C), F16, kind="ExternalInput").ap()
    wqk_ap = nc.dram_tensor("WQK", (128, HL * 8 * 128), F16,
                            kind="ExternalInput").ap()
    wv_ap = nc.dram_tensor("WV", (128, 8 * W), F16, kind="ExternalInput").ap()
    wp_ap = nc.dram_tensor("WP", (128, 4 * C), F16, kind="ExternalInput").ap()
    bias_ap = nc.dram_tensor("BIAS", (1, C), F32, kind="ExternalInput").ap()
    amask_ap = nc.dram_tensor("AMASK", (128, 4 * 512), F16,
                              kind="ExternalInput").ap()
    out_ap = nc.dram_tensor("OUT", (T, C), F32, kind="ExternalOutput").ap()

    with tile.TileContext(nc) as tc, ExitStack() as ctx:
        pers = ctx.enter_context(tc.tile_pool(name="pers", bufs=1))
        wqk_sb = pers.tile([128, HL, 8, 128], F16)
        wv_sb = pers.tile([128, 8, W], F16)
        wp_sb = pers.tile([128, 4, C], F16)
        bias_sb = pers.tile([1, C], F32R)
        amask_sb = pers.tile([128, 4, 512], F16)
        ones_row = pers.tile([1, 128], F32)
        bias_bc = pers.tile([128, C], F32)
        xT = pers.tile([128, 8, T], F16)            # xT[p, ct, t] = x[t, ct*128+p]
        v_sb = pers.tile([128, 16, HL, HS + 1], F16)  # [key-part, kc, h, d | ones]
        attn_sb = pers.tile([128, 16, HL, HS], F16)   # [t-part, tt, h, d]
        attnT = pers.tile([128, 16, 4, 128], F16)     # [w-part, tt, ct, t]

        nc.scalar.dma_start(wqk_sb, wqk_ap.rearrange(
            "p (h ct m) -> p h ct m", h=HL, ct=8))
        nc.scalar.dma_start(wv_sb, wv_ap.rearrange("p (ct w) -> p ct w", ct=8))
        nc.scalar.dma_start(wp_sb, wp_ap.rearrange("p (ct n) -> p ct n", ct=4))
        nc.scalar.dma_start(bias_sb, bias_ap.bitcast(F32R))
        nc.scalar.dma_start(amask_sb, amask_ap.rearrange(
            "p (j f) -> p j f", j=4))
        nc.vector.memset(ones_row, 1.0)
        nc.vector.memset(v_sb[:, :, :, HS:HS + 1], 1.0)

        for _rep in range(repeat):
            # ---- Phase 1: xT via XBAR dma transpose (DRAM -> SBUF) ----
            for ct in range(8):
                eng = nc.sync if ct % 2 == 0 else nc.scalar
                eng.dma_start_transpose(xT[:, ct, :],
                                        xb_ap[:, ct * 128:(ct + 1) * 128])

            # ---- Phase 0b: bias broadcast [128, C] via PE ----
            with tc.tile_pool(name="psb", bufs=2, space="PSUM") as psb:
                for chh in range(2):
                    pbb = psb.tile([128, 512], F32)
                    nc.tensor.matmul(pbb, ones_row.bitcast(F32R),
                                     bias_sb[:, chh * 512:(chh + 1) * 512],
                                     start=True, stop=True)
                    nc.vector.tensor_copy(bias_bc[:, chh * 512:(chh + 1) * 512],
                                          pbb)

            # ---- Phase 2: v = x @ Wv (natural [t, w] layout) ----
            with tc.tile_pool(name="ps2", bufs=2, space="PSUM") as ps2:
                for tt in range(16):
                    pv2 = ps2.tile([128, W], F32)
                    for ct in range(8):
                        nc.tensor.matmul(pv2, xT[:, ct, tt * 128:(tt + 1) * 128],
                                         wv_sb[:, ct, :],
                                         start=(ct == 0), stop=(ct == 7))
                    nc.vector.tensor_copy(
                        v_sb[:, tt, :, 0:HS],
                        pv2.rearrange("p (h d) -> p h d", h=HL))

            # ---- Phase 3: per-head attention ----
            with tc.tile_pool(name="qkt", bufs=2) as qkt_pool, \
                 tc.tile_pool(name="qhi", bufs=2) as qhi_pool, \
                 tc.tile_pool(name="pp", bufs=16) as p_pool, \
                 tc.tile_pool(name="nrm", bufs=4) as n_pool, \
                 tc.tile_pool(name="psq", bufs=2, space="PSUM") as ps_q, \
                 tc.tile_pool(name="pss", bufs=2, space="PSUM") as ps_s, \
                 tc.tile_pool(name="psv", bufs=2, space="PSUM") as ps_v:

                for h in range(HL):
                    # qk-proj: qkT[0:64] = q^T, qkT[64:128] = k^T
                    qkT = qkt_pool.tile([128, T], F16)
                    qhi = qhi_pool.tile([128, T], F16)
                    for tq in range(4):
                        pqk = ps_q.tile([128, 512], F32)
                        for ct in range(8):
                            nc.tensor.matmul(
                                pqk, wqk_sb[:, h, ct, :],
                                xT[:, ct, tq * 512:(tq + 1) * 512],
                                start=(ct == 0), stop=(ct == 7))
                        nc.vector.tensor_copy(
                            qkT[:, tq * 512:(tq + 1) * 512], pqk)
                        # stage q chunk into partitions 64..127 (same base
                        # partition as k^T for the scores matmul)
                        nc.sync.dma_start(qhi[64:128, tq * 512:(tq + 1) * 512],
                                          qkT[0:64, tq * 512:(tq + 1) * 512])

                    p_rows = []  # p_rows[r] = list of p tiles (2 kc each)
                    for r in range(4):
                        # scores + exp for query row r (queries 512r..512r+511)
                        tiles_r = []
                        for g in range(2 * (r + 1)):
                            sps = ps_s.tile([128, 2, 512], F32)
                            for j in range(2):
                                kc = 2 * g + j
                                nc.tensor.matmul(
                                    sps[:, j, :],
                                    qkT[64:128, kc * 128:(kc + 1) * 128],
                                    qhi[64:128, r * 512:(r + 1) * 512],
                                    start=True, stop=True)
                            p_t = p_pool.tile([128, 2, 512], F16)
                            nc.scalar.activation(p_t, sps, AF.Exp,
                                                 bias=0.0, scale=SCALE)
                            for j in range(2):
                                kc = 2 * g + j
                                if kc >= 4 * r:
                                    nc.gpsimd.tensor_mul(
                                        p_t[:, j, :], p_t[:, j, :],
                                        amask_sb[:, kc - 4 * r, :])
                            tiles_r.append(p_t)
                        p_rows.append(tiles_r)
                        if r >= 1:
                            self_pv(nc, ps_v, n_pool, p_rows[r - 1], v_sb,
                                    attn_sb, h, r - 1)
                    self_pv(nc, ps_v, n_pool, p_rows[3], v_sb, attn_sb, h, 3)

            # ---- Phase 3b: attn^T via XBAR dma transpose (SBUF -> SBUF) ----
            for tt in range(16):
                eng = nc.sync if tt % 2 == 0 else nc.scalar
                eng.dma_start_transpose(attnT[:, tt, :, :],
                                        attn_sb[:, tt, :, :])

            # ---- Phase 4: out = attn @ Wproj + bias ----
            out_r = out_ap.rearrange("(tt p) n -> p tt n", p=128)
            with tc.tile_pool(name="ostg", bufs=2) as o_pool, \
                 tc.tile_pool(name="ps4", bufs=4, space="PSUM") as ps4:
                for tt in range(16):
                    ostg = o_pool.tile([128, C], F32)
                    for chh in range(2):
                        po = ps4.tile([128, 512], F32)
                        for ct in range(4):
                            nc.tensor.matmul(
                                po, attnT[:, tt, ct, :],
                                wp_sb[:, ct, chh * 512:(chh + 1) * 512],
                                start=(ct == 0), stop=(ct == 3))
                        nc.vector.tensor_add(
                            ostg[:, chh * 512:(chh + 1) * 512], po,
                            bias_bc[:, chh * 512:(chh + 1) * 512])
                    eng = nc.sync if tt % 2 == 0 else nc.scalar
                    eng.dma_start(out_r[:, tt, :], ostg)

    nc.finalize()
    return nc


def self_pv(nc, ps_v, n_pool, tiles_r, v_sb, attn_sb, h, r):
    """PV for query row r: out[t, d] = sum_kc p[kc]^T @ [v|1], then 1/l."""
    pvq = ps_v.tile([128, 4, HS + 1], F32)  # 4 query blocks, one PSUM bank
    for qq in range(4):
        qb = 4 * r + qq
        for kc in range(qb + 1):
            p_t = tiles_r[kc // 2]
            nc.tensor.matmul(pvq[:, qq, :],
                             p_t[:, kc % 2, qq * 128:(qq + 1) * 128],
                             v_sb[:, kc, h, :],
                             start=(kc == 0), stop=(kc == qb))
    rcp4 = n_pool.tile([128, 4, 1], F32)
    nc.vector.reciprocal(rcp4, pvq[:, :, HS:HS + 1])
    for qq in range(4):
        qb = 4 * r + qq
        nc.vector.tensor_scalar_mul(attn_sb[:, qb, h, :],
                                    pvq[:, qq, 0:HS], rcp4[:, qq, :])


def _in_maps(inputs):
    x = np.asarray(inputs["x"], dtype=np.float32)
    Wq = np.asarray(inputs["Wq"], dtype=np.float32)
    Wk = np.asarray(inputs["Wk"], dtype=np.float32)
    Wv = np.asarray(inputs["Wv"], dtype=np.float32)
    Wp = np.asarray(inputs["Wproj"], dtype=np.float32)
    bp = np.asarray(inputs["bproj"], dtype=np.float32)

    s = np.arange(128)[:, None, None]
    j = np.arange(4)[None, :, None]
    tf = np.arange(512)[None, None, :]
    amask = np.where(128 * j + s > tf, np.float16(0.0), np.float16(1.0))
    amask = np.ascontiguousarray(
        amask.astype(np.float16).reshape(128, 4 * 512))

    maps = []
    for core in range(8):
        b, hh = core // 2, core % 2
        hs0 = hh * HL
        # WQK[p, (h ct m)] = [Wq|Wk][hs0+h][ct*128+p, m]
        wqk = np.concatenate([Wq[hs0:hs0 + HL], Wk[hs0:hs0 + HL]],
                             axis=2)  # [HL, C, 128]
        wqk = wqk.reshape(HL, 8, 128, 128).transpose(2, 0, 1, 3)
        wqk = np.ascontiguousarray(
            wqk.reshape(128, HL * 8 * 128).astype(np.float16))
        # WV[p, (ct w)] = Wv_cat[ct*128+p, w], w = h*64+d
        wv = Wv[hs0:hs0 + HL].transpose(1, 0, 2).reshape(C, W)
        wv = np.ascontiguousarray(
            wv.reshape(8, 128, W).transpose(1, 0, 2).reshape(128, 8 * W)
            .astype(np.float16))
        # WP[p, (ct n)] = Wp_local[ct*128+p, n]
        wp = Wp[hh * W:(hh + 1) * W]
        wp = np.ascontiguousarray(
            wp.reshape(4, 128, C).transpose(1, 0, 2).reshape(128, 4 * C)
            .astype(np.float16))
        bias = (bp if hh == 0 else np.zeros_like(bp)).reshape(1, C)
        maps.append({
            "XB": np.ascontiguousarray(x[b].astype(np.float16)),
            "WQK": wqk, "WV": wv, "WP": wp,
            "BIAS": np.ascontiguousarray(bias.astype(np.float32)),
            "AMASK": amask,
        })
    return maps


def get_nc(repeat=1):
    key = repeat
    if key not in _NC:
        _NC[key] = _build(repeat)
    return _NC[key]


def run(inputs, trace=False):
    res = bass_utils.run_bass_kernel_spmd(
        get_nc(), _in_maps(inputs), core_ids=list(range(8)), trace=trace)
    outs = [res.results[c]["OUT"] for c in range(8)]
    out = np.stack([outs[2 * b] + outs[2 * b + 1] for b in range(B)])
    return out.astype(np.float32), res.exec_time_ns


def kernel(**inputs):
    return run(inputs, trace=False)[0]


# revision 16
# speedup vs baseline: 2.2825x; 1.5430x over previous
import sys
sys.path.insert(0, "/opt/trn_rl_repo")
import numpy as np
from contextlib import ExitStack

from concourse import bacc, mybir, tile
from concourse import bass_utils

# nn_MultiHeadAttention: B=4, T=2048, C=1024, H=16, HS=64
# Sharding: core = 2*b + hh; each core handles batch b, heads hh*8..hh*8+7.
# Per-core output is a partial [T, C] (its 8 heads through Wproj rows);
# host sums the pair (hh=0,1) per batch. Bias folded into hh==0 cores.
#
# fp16 compute, f32 PSUM accumulation. Attention out kept in natural
# [t, d] layout (PV matmul p.T @ [v|1] with query on partitions), both
# transposes (x^T, attn^T) via XBAR dma_start_transpose, exp merged into
# [128, 2*512] activation instructions, all PSUM->SBUF copies on Pool/DVE.

B, T, C = 4, 2048, 1024
H, HS = 16, 64
HL = 8            # local heads per core
W = HL * HS       # 512
SCALE = 1.0 / 32.0  # C ** -0.5

F32 = mybir.dt.float32
F32R = mybir.dt.float32r
F16 = mybir.dt.float16
AF = mybir.ActivationFunctionType
ALU = mybir.AluOpType

_NC = {}


def _build(repeat=1):
    nc = bacc.Bacc("TRN2", target_bir_lowering=False, debug=False, num_devices=1)
    xb_ap = nc.dram_tensor("XB", (T, C), F16, kind="ExternalInput").ap()
    wqk_ap = nc.dram_tensor("WQK", (128, HL * 8 * 128), F16,
                            kind="ExternalInput").ap()
    wv_ap = nc.dram_tensor("WV", (128, 8 * W), F16, kind="ExternalInput").ap()
    wp_ap = nc.dram_tensor("WP", (128, 4 * C), F16, kind="ExternalInput").ap()
    bias_ap = nc.dram_tensor("BIAS", (1, C), F32, kind="ExternalInput").ap()
    amask_ap = nc.dram_tensor("AMASK", (128, 4 * 512), F16,
                              kind="ExternalInput").ap()
    out_ap = nc.dram_tensor("OUT", (T, C), F32, kind="ExternalOutput").ap()

    with tile.TileContext(nc) as tc, ExitStack() as ctx:
        pers = ctx.enter_context(tc.tile_pool(name="pers", bufs=1))
        wqk_sb = pers.tile([128, HL, 8, 128], F16)
        wv_sb = pers.tile([128, 8, W], F16)
        wp_sb = pers.tile([128, 4, C], F16)
        bias_sb = pers.tile([1, C], F32R)
        amask_sb = pers.tile([128, 4, 512], F16)
        ones_row = pers.tile([1, 128], F32)
        bias_bc = pers.tile([128, C], F32)
        xT = pers.tile([128, 8, T], F16)            # xT[p, ct, t] = x[t, ct*128+p]
        v_sb = pers.tile([128, 16, HL, HS + 1], F16)  # [key-part, kc, h, d | ones]
        attn_sb = pers.tile([128, 16, HL, HS], F16)   # [t-part, tt, h, d]
        attnT = pers.tile([128, 16, 4, 128], F16)     # [w-part, tt, ct, t]

        nc.scalar.dma_start(wqk_sb, wqk_ap.rearrange(
            "p (h ct m) -> p h ct m", h=HL, ct=8))
        nc.scalar.dma_start(wv_sb, wv_ap.rearrange("p (ct w) -> p ct w", ct=8))
        nc.scalar.dma_start(wp_sb, wp_ap.rearrange("p (ct n) -> p ct n", ct=4))
        nc.scalar.dma_start(bias_sb, bias_ap.bitcast(F32R))
        nc.scalar.dma_start(amask_sb, amask_ap.rearrange(
            "p (j f) -> p j f", j=4))
        nc.vector.memset(ones_row, 1.0)
        nc.vector.memset(v_sb[:, :, :, HS:HS + 1], 1.0)

        for _rep in range(repeat):
            # ---- Phase 1: xT via XBAR dma transpose (DRAM -> SBUF) ----
            for ct in range(8):
                eng = nc.sync if ct % 2 == 0 else nc.scalar
                eng.dma_start_transpose(xT[:, ct, :],
                                        xb_ap[:, ct * 128:(ct + 1) * 128])

            # ---- Phase 0b: bias broadcast [128, C] via PE ----
            with tc.tile_pool(name="psb", bufs=2, space="PSUM") as psb:
                for chh in range(2):
                    pbb = psb.tile([128, 512], F32)
                    nc.tensor.matmul(pbb, ones_row.bitcast(F32R),
                                     bias_sb[:, chh * 512:(chh + 1) * 512],
                                     start=True, stop=True)
                    nc.vector.tensor_copy(bias_bc[:, chh * 512:(chh + 1) * 512],
                                          pbb)

            # ---- Phase 2: v = x @ Wv (natural [t, w] layout) ----
            with tc.tile_pool(name="ps2", bufs=2, space="PSUM") as ps2:
                for tt in range(16):
                    pv2 = ps2.tile([128, W], F32)
                    for ct in range(8):
                        nc.tensor.matmul(pv2, xT[:, ct, tt * 128:(tt + 1) * 128],
                                         wv_sb[:, ct, :],
                                         start=(ct == 0), stop=(ct == 7))
                    nc.vector.tensor_copy(
                        v_sb[:, tt, :, 0:HS],
                        pv2.rearrange("p (h d) -> p h d", h=HL))

            # ---- Phase 3: per-head attention ----
            with tc.tile_pool(name="qkt", bufs=2) as qkt_pool, \
                 tc.tile_pool(name="qhi", bufs=2) as qhi_pool, \
                 tc.tile_pool(name="pp", bufs=16) as p_pool, \
                 tc.tile_pool(name="nrm", bufs=4) as n_pool, \
                 tc.tile_pool(name="psq", bufs=2, space="PSUM") as ps_q, \
                 tc.tile_pool(name="pss", bufs=2, space="PSUM") as ps_s, \
                 tc.tile_pool(name="psv", bufs=2, space="PSUM") as ps_v:

                for h in range(HL):
                    # qk-proj: qkT[0:64] = q^T, qkT[64:128] = k^T
                    qkT = qkt_pool.tile([128, T], F16)
                    qhi = qhi_pool.tile([128, T], F16)
                    for tq in range(4):
                        pqk = ps_q.tile([128, 512], F32)
                        for ct in range(8):
                            nc.tensor.matmul(
                                pqk, wqk_sb[:, h, ct, :],
                                xT[:, ct, tq * 512:(tq + 1) * 512],
                                start=(ct == 0), stop=(ct == 7))
                        nc.vector.tensor_copy(
                            qkT[:, tq * 512:(tq + 1) * 512], pqk)
                        # stage q chunk into partitions 64..127 (same base
                        # partition as k^T for the scores matmul)
                        nc.sync.dma_start(qhi[64:128, tq * 512:(tq + 1) * 512],
                                          qkT[0:64, tq * 512:(tq + 1) * 512])

                    p_rows = []  # p_rows[r] = list of p tiles (2 kc each)
                    for r in range(4):
                        # scores + exp for query row r (queries 512r..512r+511)
                        # Diagonal kc chunks (kc >= 4r) are trimmed to the
                        # causal triangle: only queries >= 128*(kc-4r) are
                        # computed/exp'd; the sub-diagonal remainder of the
                        # p tile is never read by PV. The fine [128,128]
                        # triangle at the diagonal is masked on DVE.
                        tiles_r = []
                        for g in range(2 * (r + 1)):
                            sps = ps_s.tile([128, 2, 512], F32)
                            p_t = p_pool.tile([128, 2, 512], F16)
                            for j in range(2):
                                kc = 2 * g + j
                                off = max(0, (kc - 4 * r) * 128)
                                nc.tensor.matmul(
                                    sps[:, j, off:512],
                                    qkT[64:128, kc * 128:(kc + 1) * 128],
                                    qhi[64:128, r * 512 + off:(r + 1) * 512],
                                    start=True, stop=True)
                            offs = [max(0, (2 * g + j - 4 * r) * 128)
                                    for j in range(2)]
                            if offs[0] == offs[1]:
                                nc.scalar.activation(p_t, sps, AF.Exp,
                                                     bias=0.0, scale=SCALE)
                            else:
                                for j in range(2):
                                    nc.scalar.activation(
                                        p_t[:, j, offs[j]:512],
                                        sps[:, j, offs[j]:512],
                                        AF.Exp, bias=0.0, scale=SCALE)
                            for j in range(2):
                                kc = 2 * g + j
                                if kc >= 4 * r:
                                    off = (kc - 4 * r) * 128
                                    nc.vector.tensor_mul(
                                        p_t[:, j, off:off + 128],
                                        p_t[:, j, off:off + 128],
                                        amask_sb[:, 0, 0:128])
                            tiles_r.append(p_t)
                        p_rows.append(tiles_r)
                        if r >= 1:
                            self_pv(nc, ps_v, n_pool, p_rows[r - 1], v_sb,
                                    attn_sb, h, r - 1)
                    self_pv(nc, ps_v, n_pool, p_rows[3], v_sb, attn_sb, h, 3)

            # ---- Phase 3b: attn^T via XBAR dma transpose (SBUF -> SBUF) ----
            for tt in range(16):
                eng = nc.sync if tt % 2 == 0 else nc.scalar
                eng.dma_start_transpose(attnT[:, tt, :, :],
                                        attn_sb[:, tt, :, :])

            # ---- Phase 4: out = attn @ Wproj + bias ----
            out_r = out_ap.rearrange("(tt p) n -> p tt n", p=128)
            with tc.tile_pool(name="ostg", bufs=2) as o_pool, \
                 tc.tile_pool(name="ps4", bufs=4, space="PSUM") as ps4:
                for tt in range(16):
                    ostg = o_pool.tile([128, C], F32)
                    for chh in range(2):
                        po = ps4.tile([128, 512], F32)
                        for ct in range(4):
                            nc.tensor.matmul(
                                po, attnT[:, tt, ct, :],
                                wp_sb[:, ct, chh * 512:(chh + 1) * 512],
                                start=(ct == 0), stop=(ct == 3))
                        nc.vector.tensor_add(
                            ostg[:, chh * 512:(chh + 1) * 512], po,
                            bias_bc[:, chh * 512:(chh + 1) * 512])
                    eng = nc.sync if tt % 2 == 0 else nc.scalar
                    eng.dma_start(out_r[:, tt, :], ostg)

    nc.finalize()
    return nc


def self_pv(nc, ps_v, n_pool, tiles_r, v_sb, attn_sb, h, r):
    """PV for query row r: out[t, d] = sum_kc p[kc]^T @ [v|1], then 1/l."""
    pvq = ps_v.tile([128, 4, HS + 1], F32)  # 4 query blocks, one PSUM bank
    for qq in range(4):
        qb = 4 * r + qq
        for kc in range(qb + 1):
            p_t = tiles_r[kc // 2]
            nc.tensor.matmul(pvq[:, qq, :],
                             p_t[:, kc % 2, qq * 128:(qq + 1) * 128],
                             v_sb[:, kc, h, :],
                             start=(kc == 0), stop=(kc == qb))
    rcp4 = n_pool.tile([128, 4, 1], F32)
    nc.vector.reciprocal(rcp4, pvq[:, :, HS:HS + 1])
    for qq in range(4):
        qb = 4 * r + qq
        nc.vector.tensor_scalar_mul(attn_sb[:, qb, h, :],
                                    pvq[:, qq, 0:HS], rcp4[:, qq, :])


def _in_maps(inputs):
    x = np.asarray(inputs["x"], dtype=np.float32)
    Wq = np.asarray(inputs["Wq"], dtype=np.float32)
    Wk = np.asarray(inputs["Wk"], dtype=np.float32)
    Wv = np.asarray(inputs["Wv"], dtype=np.float32)
    Wp = np.asarray(inputs["Wproj"], dtype=np.float32)
    bp = np.asarray(inputs["bproj"], dtype=np.float32)

    s = np.arange(128)[:, None, None]
    j = np.arange(4)[None, :, None]
    tf = np.arange(512)[None, None, :]
    amask = np.where(128 * j + s > tf, np.float16(0.0), np.float16(1.0))
    amask = np.ascontiguousarray(
        amask.astype(np.float16).reshape(128, 4 * 512))

    maps = []
    for core in range(8):
        b, hh = core // 2, core % 2
        hs0 = hh * HL
        # WQK[p, (h ct m)] = [Wq|Wk][hs0+h][ct*128+p, m]
        wqk = np.concatenate([Wq[hs0:hs0 + HL], Wk[hs0:hs0 + HL]],
                             axis=2)  # [HL, C, 128]
        wqk = wqk.reshape(HL, 8, 128, 128).transpose(2, 0, 1, 3)
        wqk = np.ascontiguousarray(
            wqk.reshape(128, HL * 8 * 128).astype(np.float16))
        # WV[p, (ct w)] = Wv_cat[ct*128+p, w], w = h*64+d
        wv = Wv[hs0:hs0 + HL].transpose(1, 0, 2).reshape(C, W)
        wv = np.ascontiguousarray(
            wv.reshape(8, 128, W).transpose(1, 0, 2).reshape(128, 8 * W)
            .astype(np.float16))
        # WP[p, (ct n)] = Wp_local[ct*128+p, n]
        wp = Wp[hh * W:(hh + 1) * W]
        wp = np.ascontiguousarray(
            wp.reshape(4, 128, C).transpose(1, 0, 2).reshape(128, 4 * C)
            .astype(np.float16))
        bias = (bp if hh == 0 else np.zeros_like(bp)).reshape(1, C)
        maps.append({
            "XB": np.ascontiguousarray(x[b].astype(np.float16)),
            "WQK": wqk, "WV": wv, "WP": wp,
            "BIAS": np.ascontiguousarray(bias.astype(np.float32)),
            "AMASK": amask,
        })
    return maps


def get_nc(repeat=1):
    key = repeat
    if key not in _NC:
        _NC[key] = _build(repeat)
    return _NC[key]


def run(inputs, trace=False):
    res = bass_utils.run_bass_kernel_spmd(
        get_nc(), _in_maps(inputs), core_ids=list(range(8)), trace=trace)
    outs = [res.results[c]["OUT"] for c in range(8)]
    out = np.stack([outs[2 * b] + outs[2 * b + 1] for b in range(B)])
    return out.astype(np.float32), res.exec_time_ns


def kernel(**inputs):
    return run(inputs, trace=False)[0]
